# revision 9
# baseline (speedup 1.0000x reference)
"""Binarized linear: out = sign(x+eps) @ sign(w+eps).T on 8 trn2 cores.

Sharding: 4x2 grid. Core c=(r,s): rows x[r*2048:(r+1)*2048], rows
w[s*2048:(s+1)*2048]. Each core computes a [2048, 2048] output block; the
host concatenates. No collectives.

Per-core kernel (all arithmetic exact -> rel err 0 vs the f32 reference):
  - binarize BOTH operands to fp8e4m3 +/-0.5 on DVE ((v>=0)-0.5, matching
    sign(v+1e-20) away from the measure-zero region f32 randn never hits).
    Products are +/-0.25 -> output eviction scales PSUM by 4: exact even
    integers |v|<=4096, stored f16 (exact; host casts back to f32).
  - transpose to [K-on-partition] layout via PE is_transpose matmuls on
    fp16 PAIRS (two adjacent fp8 k-values ride one 16-bit lane; all our
    fp8 byte pairs form normal f16 values so the move is bit-exact), TG=4
    tiles per PSUM group, one contiguous f16 eviction per group (ACT).
    (A DMA-XBAR variant was measured: transposes inflate 3->9us when
    competing with input loads for the 16 DMA engines, and the shared
    DMA-semaphore rotation couples input loads to XBAR completions,
    collapsing supply to ~250GB/s. PE transposes pump into the DR stream's
    inter-pass bubbles at ~110ns net each instead.)
  - DR matmuls run perf_mode=DoubleRowSwInterleave on the interleaved pair
    layout (hw deinterleaves; reads the stationary m axis reversed -- host
    un-flips each 128-row output block). Measured steady cadence: 259ns
    per K=256 x 512-wide pass; the ~46ns/pass gap vs 213ns streaming
    theory is a fixed per-instruction bubble (PSUM-bank alternation and
    2-bank-wide outputs measured/rejected: no effect / ISA-illegal).
  - floors: PE 1024 DR passes x 259ns = 265us/core + ~35-55us net
    transposes; DMA 64MB input at ~380GB/s one-queue serial (16 engines
    x ~24GB/s each, saturated by a single queue's 16KB-row descriptors).

Queues: SP (sync) = input loads only, one 2MB full-row DMA per 128-row
block, in arrival order: w-jb0, x0, w-jb1..3, x1, then x blocks 1:1 with
w jbs. ACT (scalar) = transpose-group evictions + output stores.

Schedule ("debt" order -- first matmul after only ~4MB has landed, vs 20MB
for the v1 two-phase schedule, and supply stays under the ~380GB/s ceiling
in every phase, where v1's x-streaming phase demanded ~360+):
  - narrow phase (~t=16us): 8 n=128 pass-sets (ib 0/1 x jb 0..3) as soon
    as w-jb0+x0 are transposed, covering (ib0,jc0)+(ib1,jc0). Warm
    matmuls (no-dep identity passes) pad the PE clock gate open.
  - solo: mm(ib,0) ib 2..5 (x-arrival paced) while chunk1 lands.
  - pairs: mm(ib,0)+mm(ib,1) ib 6..15; chunks 2/3 stream behind.
  - debt: owed mm(0..5,1), then jc=2, jc=3 sweeps (all resident).
  - transpose groups pump one per DR pass; out evictions (DVE) are
    emitted one set late so their PE-completion waits never
    head-of-line-block the DVE queue (which also binarizes).
The Tile scheduler is fed PE timings scaled 2x (build_program patches
TRN2Spec) because the stock cost model prices DR fp8 matmuls at half
their measured hardware cost.
"""

from collections import deque

import numpy as np

P = 128
GRID_I, GRID_J = 4, 2
N_CORES = 8
FULL_M, FULL_N, FULL_K = 8192, 4096, 4096
M_SH, N_SH = FULL_M // GRID_I, FULL_N // GRID_J  # 2048, 2048

_PROGRAM_CACHE = {}


def build_program(m_sh=M_SH, n_sh=N_SH, k=FULL_K, warmup=64, out_fp16=True):
    """Build (and cache) the per-core Bass program. Same SPMD program on all cores."""
    key = (m_sh, n_sh, k, warmup, out_fp16)
    if key in _PROGRAM_CACHE:
        return _PROGRAM_CACHE[key]

    from contextlib import ExitStack

    import concourse.bass as bass
    import concourse.mybir as mybir
    from concourse import bacc, tile
    from concourse.masks import make_identity

    # Feed the Tile scheduler PE timings that match measured hw (stock model
    # prices DR fp8 at 0.5 cyc/row; hw runs ~1.21 cyc/row incl the bubble).
    from concourse import hw_specs as _hw
    _hw.TRN2Spec.PE_CYCLE = 2.0 / 2.4
    _hw.TRN2Spec.PE_CYCLE_PSTATE_MID = 2.0 / 1.2
    _hw.TRN2Spec.PE_CYCLE_PSTATE_LOW = 2.0 / 0.65

    f32 = mybir.dt.float32
    f16 = mybir.dt.float16
    fp8 = mybir.dt.float8e4
    out_dt = f16 if out_fp16 else f32

    KT16 = k // (2 * P)  # 128-wide f16-pair k tiles (16)
    IB = m_sh // P       # 16 x blocks
    JBLK = 512
    JC = n_sh // JBLK    # 4 w chunks
    JB = n_sh // P       # 16 w j-blocks
    JB_PER_JC = JBLK // P
    TG = 4               # f16 tiles per transpose-evict group
    assert KT16 % TG == 0

    DR = mybir.MatmulPerfMode.DoubleRowSwInterleave

    nc = bacc.Bacc("TRN2", target_bir_lowering=False, debug=False)
    xs = nc.dram_tensor("xs", [m_sh, k], f32, kind="ExternalInput").ap()
    ws = nc.dram_tensor("ws", [n_sh, k], f32, kind="ExternalInput").ap()
    out = nc.dram_tensor("out", [m_sh, n_sh], out_dt, kind="ExternalOutput").ap()

    with tile.TileContext(nc) as tc, ExitStack() as ctx:
        const_pool = ctx.enter_context(tc.tile_pool(name="const", bufs=1))
        stage = ctx.enter_context(tc.tile_pool(name="stage", bufs=3))
        xb8_pool = ctx.enter_context(tc.tile_pool(name="xb8", bufs=2))
        wb8_pool = ctx.enter_context(tc.tile_pool(name="wb8", bufs=2))
        xbt_pool = ctx.enter_context(tc.tile_pool(name="xbt", bufs=1))
        wbt_pool = ctx.enter_context(tc.tile_pool(name="wbt", bufs=1))
        out_pool = ctx.enter_context(tc.tile_pool(name="outp", bufs=3))
        outn_pool = ctx.enter_context(tc.tile_pool(name="outn", bufs=2))
        psum_t = ctx.enter_context(tc.tile_pool(name="pst", bufs=2, space="PSUM"))
        psum_mm = ctx.enter_context(tc.tile_pool(name="psmm", bufs=3, space="PSUM"))
        psum_nr = ctx.enter_context(tc.tile_pool(name="psnr", bufs=2, space="PSUM"))
        psum_wu = ctx.enter_context(tc.tile_pool(name="pswu", bufs=1, space="PSUM"))

        ident = const_pool.tile([P, P], fp8, tag="ident")
        make_identity(nc, ident)
        ident16 = const_pool.tile([P, P], f16, tag="ident16")
        nc.vector.tensor_copy(ident16[:], ident[:])

        warm_psum = psum_wu.tile([P, P], f32, tag="warm", name="warm") if warmup else None

        def warm(n):
            for _ in range(n):
                nc.tensor.matmul(warm_psum[:], lhsT=ident[:], rhs=ident[:],
                                 start=True, stop=True)

        if warmup:
            warm(warmup)

        # Resident transposed binarized operands, f16 pair layout, K on
        # partitions: f16 k-tile kp, partition p, byte b holds fp8 k value
        # 256*kp + 2p + b (consistent across x and w).
        xbT = [
            xbt_pool.tile([P, KT16, P], f16, tag=f"xbt{ib}", name=f"xbt{ib}")
            for ib in range(IB)
        ]
        wbT = [
            wbt_pool.tile([P, KT16, JBLK], f16, tag=f"wbt{jc}", name=f"wbt{jc}")
            for jc in range(JC)
        ]

        def binarize(b8h, stgh):
            # (v >= 0) -> {1,0}; minus 0.5 -> +/-0.5.
            nc.vector.tensor_scalar(
                b8h, stgh, 0.0, 0.5,
                mybir.AluOpType.is_ge, mybir.AluOpType.subtract,
            )

        pending = deque()  # (key, closure): transpose groups, 4 PE transposes each
        pending_cnt = {}   # key -> groups still pending

        def queue_tgroups(b8, dest, dest_col0, key):
            """Transpose b8 (fp8 [128, k]) into dest[:, :, col0:col0+P] (f16
            pair layout) via fp16-pair PE transposes + one contiguous f16
            eviction per group (ACT)."""
            for g in range(KT16 // TG):
                def go(g=g, b8=b8, dest=dest, dest_col0=dest_col0):
                    pt = psum_t.tile([P, TG, P], f16, tag="pt", name="pt")
                    for t in range(TG):
                        t16 = g * TG + t
                        in16 = b8[:, 2 * P * t16:2 * P * (t16 + 1)].bitcast(f16)
                        nc.tensor.transpose(pt[:, t, :], in16, ident16[:])
                    nc.scalar.copy(
                        dest[:, TG * g:TG * (g + 1), dest_col0:dest_col0 + P],
                        pt[:])
                pending.append((key, go))
                pending_cnt[key] = pending_cnt.get(key, 0) + 1

        def pump(n, hold=0):
            """Emit up to n pending transpose groups (oldest first); with
            hold>0, keep at least that many queued (so a pumped transpose's
            input load was issued several blocks back and its data-wait
            never stalls the PE FIFO)."""
            for _ in range(n):
                if len(pending) <= hold:
                    return
                key, go = pending.popleft()
                pending_cnt[key] -= 1
                go()

        prepped = set()

        def require(key):
            """Emit all transpose groups up to and including key's (FIFO
            order matches need order). Must precede any matmul reading the
            operand tile that key's groups write."""
            assert key in prepped, f"operand {key} never prepped"
            while pending_cnt.get(key, 0) > 0:
                key2, go = pending.popleft()
                pending_cnt[key2] -= 1
                go()

        def prep_x(ib):
            stg = stage.tile([P, k], f32, tag="stg", name="stg")
            nc.sync.dma_start(stg[:], xs[ib * P:(ib + 1) * P, :])
            b8 = xb8_pool.tile([P, k], fp8, tag="xb8", name="xb8")
            binarize(b8[:], stg[:])
            queue_tgroups(b8, xbT[ib], 0, ("x", ib))
            prepped.add(("x", ib))

        def prep_w_jb(jb):
            jc, sub = divmod(jb, JB_PER_JC)
            stg = stage.tile([P, k], f32, tag="stg", name="stg")
            nc.sync.dma_start(stg[:], ws[jb * P:(jb + 1) * P, :])
            b8 = wb8_pool.tile([P, k], fp8, tag="wb8", name="wb8")
            binarize(b8[:], stg[:])
            queue_tgroups(b8, wbT[jc], sub * P, ("w", jb))
            prepped.add(("w", jb))

        # ---- load pacing: interleave remaining x blocks and w jbs ----
        load_q = []
        for i in range(2, IB):
            load_q.append(("x", i))
            if 2 + i < JB:
                load_q.append(("w", 2 + i))
        for j in range(2 + IB, JB):
            load_q.append(("w", j))

        def advance_loads(n):
            for _ in range(n):
                if not load_q:
                    return
                kind, idx = load_q.pop(0)
                if kind == "x":
                    prep_x(idx)
                else:
                    prep_w_jb(idx)

        outq = []

        def emit_out(ps, ib, jc, width):
            ob_pool, tag = (out_pool, "ob") if width == JBLK else (outn_pool, "obn")
            ob = ob_pool.tile([P, width], out_dt, tag=tag, name=tag)
            # products are +/-0.5 * +/-0.5 = +/-0.25 -> scale by 4
            nc.vector.tensor_scalar_mul(ob[:], ps[:], 4.0)
            nc.scalar.dma_start(
                out[ib * P:(ib + 1) * P, jc * width:(jc + 1) * width], ob[:]
            )

        def flush_out():
            while outq:
                emit_out(*outq.pop(0))

        def mm(ib, jc):
            require(("x", ib))
            for s in range(JB_PER_JC):
                require(("w", jc * JB_PER_JC + s))
            ps = psum_mm.tile([P, JBLK], f32, tag="ps", name="ps")
            for kp in range(KT16):
                lhsT = xbT[ib][:, kp, :].bitcast(fp8)
                rhs = wbT[jc][:, kp, :].bitcast(fp8).rearrange(
                    "p (n two) -> p two n", two=2)
                nc.tensor.matmul(ps[:], lhsT=lhsT, rhs=rhs,
                                 start=(kp == 0), stop=(kp == KT16 - 1),
                                 perf_mode=DR)
                pump(1, hold=8)
            if outq:
                emit_out(*outq.pop(0))
            outq.append((ps, ib, jc, JBLK))

        def narrow_mm(ib, jb):
            require(("x", ib))
            require(("w", jb))
            ps = psum_nr.tile([P, P], f32, tag="psn", name="psn")
            for kp in range(KT16):
                lhsT = xbT[ib][:, kp, :].bitcast(fp8)
                rhs = wbT[0][:, kp, jb * P:(jb + 1) * P].bitcast(fp8).rearrange(
                    "p (n two) -> p two n", two=2)
                nc.tensor.matmul(ps[:], lhsT=lhsT, rhs=rhs,
                                 start=(kp == 0), stop=(kp == KT16 - 1),
                                 perf_mode=DR)
                pump(1, hold=8)
            if outq:
                emit_out(*outq.pop(0))
            outq.append((ps, ib, jb, P))

        # ---- startup: minimal front-load; first narrow set needs only
        # w-jb0 + x0 binarized + transposed (~4MB landed).
        prep_w_jb(0)
        prep_x(0)
        prep_w_jb(1)
        prep_w_jb(2)
        prep_w_jb(3)
        prep_x(1)
        # Drain startup transpose groups with warm filler to hold the PE
        # clock gate open while the loads land.
        n_drain = len(pending)
        for i in range(n_drain):
            pump(1)
            if warmup:
                warm(6)
        for ib in (0, 1):
            for jb in range(JB_PER_JC):
                narrow_mm(ib, jb)
                if warmup and ib == 0:
                    warm(8)
        # ---- solo phase: x-arrival paced, chunk1 streaming behind.
        for ib in range(2, 6):
            advance_loads(2)
            mm(ib, 0)
        # ---- pair phase.
        for ib in range(6, IB):
            advance_loads(2)
            mm(ib, 0)
            advance_loads(1)
            mm(ib, 1)
        # ---- debt: owed (0..5, 1).
        for ib in range(6):
            advance_loads(2)
            mm(ib, 1)
        # ---- jc 2/3 sweeps; finish remaining loads early in jc2.
        for jc in range(2, JC):
            for ib in range(IB):
                advance_loads(2)
                mm(ib, jc)
        flush_out()
        pump(len(pending))

    nc.compile()
    _PROGRAM_CACHE[key] = nc
    return nc


def kernel(x, weight):
    x = np.ascontiguousarray(np.asarray(x), dtype=np.float32)
    w = np.ascontiguousarray(np.asarray(weight), dtype=np.float32)
    assert x.shape == (FULL_M, FULL_K) and w.shape == (FULL_N, FULL_K)

    from concourse.bass_utils import run_bass_kernel_spmd

    nc = build_program()
    in_maps = []
    for c in range(N_CORES):
        r, s = divmod(c, GRID_J)
        in_maps.append({
            "xs": x[r * M_SH:(r + 1) * M_SH],
            "ws": w[s * N_SH:(s + 1) * N_SH],
        })
    res = run_bass_kernel_spmd(nc, in_maps, core_ids=list(range(N_CORES))).results
    outp = np.empty((FULL_M, FULL_N), dtype=np.float32)
    for c in range(N_CORES):
        r, s = divmod(c, GRID_J)
        blk = np.asarray(res[c]["out"], dtype=np.float32)
        # SwInterleave reads the stationary m axis reversed: un-flip each
        # 128-row output block.
        blk = blk.reshape(M_SH // P, P, N_SH)[:, ::-1, :].reshape(M_SH, N_SH)
        outp[r * M_SH:(r + 1) * M_SH, s * N_SH:(s + 1) * N_SH] = blk
    return outp


# revision 13
# speedup vs baseline: 1.0947x; 1.0947x over previous
"""Binarized linear: out = sign(x+eps) @ sign(w+eps).T on 8 trn2 cores.

Sharding: 4x2 grid. Core c=(r,s): rows x[r*2048:(r+1)*2048], rows
w[s*2048:(s+1)*2048]. Each core computes a [2048, 2048] output block; the
host concatenates. No collectives.

Per-core kernel (all arithmetic exact -> rel err 0 vs the f32 reference):
  - binarize BOTH operands to fp8e4m3 +/-0.5 on DVE ((v>=0)-0.5, matching
    sign(v+1e-20) away from the measure-zero region f32 randn never hits).
    Products are +/-0.25 -> output eviction scales PSUM by 4: exact even
    integers |v|<=4096, stored f16 (exact; host casts back to f32).
  - transpose to [K-on-partition] layout via PE is_transpose matmuls on
    fp16 PAIRS (two adjacent fp8 k-values ride one 16-bit lane; all our
    fp8 byte pairs form normal f16 values so the move is bit-exact), TG=4
    tiles per PSUM group, one contiguous f16 eviction per group (ACT).
    (A DMA-XBAR variant was measured: transposes inflate 3->9us when
    competing with input loads for the 16 DMA engines, and the shared
    DMA-semaphore rotation couples input loads to XBAR completions,
    collapsing supply to ~250GB/s. PE transposes pump into the DR stream's
    inter-pass bubbles at ~110ns net each instead.)
  - DR matmuls run perf_mode=DoubleRowSwInterleave on the interleaved pair
    layout (hw deinterleaves; reads the stationary m axis reversed -- host
    un-flips each 128-row output block). Measured steady cadence: 259ns
    per K=256 x 512-wide pass; the ~46ns/pass gap vs 213ns streaming
    theory is a fixed per-instruction bubble (PSUM-bank alternation and
    2-bank-wide outputs measured/rejected: no effect / ISA-illegal).
  - floors: PE 1024 DR passes x 259ns = 265us/core + ~35-55us net
    transposes; DMA 64MB input at ~380GB/s one-queue serial (16 engines
    x ~24GB/s each, saturated by a single queue's 16KB-row descriptors).

Queues: SP (sync) = input loads only, one 2MB full-row DMA per 128-row
block, in arrival order: w-jb0, x0, w-jb1..3, x1, then x blocks 1:1 with
w jbs. ACT (scalar) = transpose-group evictions + output stores.

Schedule ("debt" order -- first matmul after only ~4MB has landed, vs 20MB
for the v1 two-phase schedule, and supply stays under the ~380GB/s ceiling
in every phase, where v1's x-streaming phase demanded ~360+):
  - narrow phase (~t=16us): 8 n=128 pass-sets (ib 0/1 x jb 0..3) as soon
    as w-jb0+x0 are transposed, covering (ib0,jc0)+(ib1,jc0). Warm
    matmuls (no-dep identity passes) pad the PE clock gate open.
  - solo: mm(ib,0) ib 2..5 (x-arrival paced) while chunk1 lands.
  - pairs: mm(ib,0)+mm(ib,1) ib 6..15; chunks 2/3 stream behind.
  - debt: owed mm(0..5,1), then jc=2, jc=3 sweeps (all resident).
  - transpose groups pump one per DR pass; out evictions (DVE) are
    emitted one set late so their PE-completion waits never
    head-of-line-block the DVE queue (which also binarizes).
The Tile scheduler is fed PE timings scaled 2x (build_program patches
TRN2Spec) because the stock cost model prices DR fp8 matmuls at half
their measured hardware cost.
"""

from collections import deque

import numpy as np

P = 128
GRID_I, GRID_J = 4, 2
N_CORES = 8
FULL_M, FULL_N, FULL_K = 8192, 4096, 4096
M_SH, N_SH = FULL_M // GRID_I, FULL_N // GRID_J  # 2048, 2048

_PROGRAM_CACHE = {}


def build_program(m_sh=M_SH, n_sh=N_SH, k=FULL_K, warmup=64, out_fp16=True):
    """Build (and cache) the per-core Bass program. Same SPMD program on all cores."""
    key = (m_sh, n_sh, k, warmup, out_fp16)
    if key in _PROGRAM_CACHE:
        return _PROGRAM_CACHE[key]

    from contextlib import ExitStack

    import concourse.bass as bass
    import concourse.mybir as mybir
    from concourse import bacc, tile
    from concourse.masks import make_identity

    # Feed the Tile scheduler PE timings that match measured hw (stock model
    # prices DR fp8 at 0.5 cyc/row; hw runs ~1.21 cyc/row incl the bubble).
    from concourse import hw_specs as _hw
    _hw.TRN2Spec.PE_CYCLE = 2.0 / 2.4
    _hw.TRN2Spec.PE_CYCLE_PSTATE_MID = 2.0 / 1.2
    _hw.TRN2Spec.PE_CYCLE_PSTATE_LOW = 2.0 / 0.65

    f32 = mybir.dt.float32
    f16 = mybir.dt.float16
    fp8 = mybir.dt.float8e4
    out_dt = f16 if out_fp16 else f32

    KT16 = k // (2 * P)  # 128-wide f16-pair k tiles (16)
    IB = m_sh // P       # 16 x blocks
    JBLK = 512
    JC = n_sh // JBLK    # 4 w chunks
    JB = n_sh // P       # 16 w j-blocks
    JB_PER_JC = JBLK // P
    TG = 4               # f16 tiles per transpose-evict group
    assert KT16 % TG == 0

    DR = mybir.MatmulPerfMode.DoubleRowSwInterleave

    nc = bacc.Bacc("TRN2", target_bir_lowering=False, debug=False)
    xs = nc.dram_tensor("xs", [m_sh, k], f32, kind="ExternalInput").ap()
    ws = nc.dram_tensor("ws", [n_sh, k], f32, kind="ExternalInput").ap()
    out = nc.dram_tensor("out", [m_sh, n_sh], out_dt, kind="ExternalOutput").ap()

    with tile.TileContext(nc) as tc, ExitStack() as ctx:
        const_pool = ctx.enter_context(tc.tile_pool(name="const", bufs=1))
        stage = ctx.enter_context(tc.tile_pool(name="stage", bufs=3))
        xb8_pool = ctx.enter_context(tc.tile_pool(name="xb8", bufs=2))
        wb8_pool = ctx.enter_context(tc.tile_pool(name="wb8", bufs=2))
        xbt_pool = ctx.enter_context(tc.tile_pool(name="xbt", bufs=1))
        wbt_pool = ctx.enter_context(tc.tile_pool(name="wbt", bufs=1))
        out_pool = ctx.enter_context(tc.tile_pool(name="outp", bufs=3))
        outn_pool = ctx.enter_context(tc.tile_pool(name="outn", bufs=2))
        psum_t = ctx.enter_context(tc.tile_pool(name="pst", bufs=2, space="PSUM"))
        psum_mm = ctx.enter_context(tc.tile_pool(name="psmm", bufs=5, space="PSUM"))
        psum_wu = ctx.enter_context(tc.tile_pool(name="pswu", bufs=1, space="PSUM"))

        ident = const_pool.tile([P, P], fp8, tag="ident")
        make_identity(nc, ident)
        ident16 = const_pool.tile([P, P], f16, tag="ident16")
        nc.vector.tensor_copy(ident16[:], ident[:])

        warm_psum = psum_wu.tile([P, P], f32, tag="warm", name="warm") if warmup else None

        def warm(n):
            for _ in range(n):
                nc.tensor.matmul(warm_psum[:], lhsT=ident[:], rhs=ident[:],
                                 start=True, stop=True)

        if warmup:
            warm(warmup)

        # Resident transposed binarized operands, f16 pair layout, K on
        # partitions: f16 k-tile kp, partition p, byte b holds fp8 k value
        # 256*kp + 2p + b (consistent across x and w).
        xbT = [
            xbt_pool.tile([P, KT16, P], f16, tag=f"xbt{ib}", name=f"xbt{ib}")
            for ib in range(IB)
        ]
        wbT = [
            wbt_pool.tile([P, KT16, JBLK], f16, tag=f"wbt{jc}", name=f"wbt{jc}")
            for jc in range(JC)
        ]

        def binarize(b8h, stgh):
            # (v >= 0) -> {1,0}; minus 0.5 -> +/-0.5.
            nc.vector.tensor_scalar(
                b8h, stgh, 0.0, 0.5,
                mybir.AluOpType.is_ge, mybir.AluOpType.subtract,
            )

        pending = deque()  # (key, closure): transpose groups, 4 PE transposes each
        pending_cnt = {}   # key -> groups still pending

        def queue_tgroups(b8, dest, dest_col0, key):
            """Transpose b8 (fp8 [128, k]) into dest[:, :, col0:col0+P] (f16
            pair layout) via fp16-pair PE transposes + one contiguous f16
            eviction per group (ACT)."""
            for g in range(KT16 // TG):
                def go(g=g, b8=b8, dest=dest, dest_col0=dest_col0):
                    pt = psum_t.tile([P, TG, P], f16, tag="pt", name="pt")
                    for t in range(TG):
                        t16 = g * TG + t
                        in16 = b8[:, 2 * P * t16:2 * P * (t16 + 1)].bitcast(f16)
                        nc.tensor.transpose(pt[:, t, :], in16, ident16[:])
                    nc.scalar.copy(
                        dest[:, TG * g:TG * (g + 1), dest_col0:dest_col0 + P],
                        pt[:])
                pending.append((key, go))
                pending_cnt[key] = pending_cnt.get(key, 0) + 1

        def pump(n, hold=0):
            """Emit up to n pending transpose groups (oldest first); with
            hold>0, keep at least that many queued (so a pumped transpose's
            input load was issued several blocks back and its data-wait
            never stalls the PE FIFO)."""
            for _ in range(n):
                if len(pending) <= hold:
                    return
                key, go = pending.popleft()
                pending_cnt[key] -= 1
                go()

        prepped = set()

        def require(key):
            """Emit all transpose groups up to and including key's (FIFO
            order matches need order). Must precede any matmul reading the
            operand tile that key's groups write."""
            assert key in prepped, f"operand {key} never prepped"
            while pending_cnt.get(key, 0) > 0:
                key2, go = pending.popleft()
                pending_cnt[key2] -= 1
                go()

        def prep_x(ib):
            stg = stage.tile([P, k], f32, tag="stg", name="stg")
            nc.sync.dma_start(stg[:], xs[ib * P:(ib + 1) * P, :])
            b8 = xb8_pool.tile([P, k], fp8, tag="xb8", name="xb8")
            binarize(b8[:], stg[:])
            queue_tgroups(b8, xbT[ib], 0, ("x", ib))
            prepped.add(("x", ib))

        def prep_w_jb(jb):
            jc, sub = divmod(jb, JB_PER_JC)
            stg = stage.tile([P, k], f32, tag="stg", name="stg")
            nc.sync.dma_start(stg[:], ws[jb * P:(jb + 1) * P, :])
            b8 = wb8_pool.tile([P, k], fp8, tag="wb8", name="wb8")
            binarize(b8[:], stg[:])
            queue_tgroups(b8, wbT[jc], sub * P, ("w", jb))
            prepped.add(("w", jb))

        # ---- load pacing: interleave remaining x blocks and w jbs ----
        load_q = []
        for i in range(2, IB):
            load_q.append(("x", i))
            if 2 + i < JB:
                load_q.append(("w", 2 + i))
        for j in range(2 + IB, JB):
            load_q.append(("w", j))

        def advance_loads(n):
            for _ in range(n):
                if not load_q:
                    return
                kind, idx = load_q.pop(0)
                if kind == "x":
                    prep_x(idx)
                else:
                    prep_w_jb(idx)

        outq = []

        def emit_out(ps, ib, jc, width):
            ob_pool, tag = (out_pool, "ob") if width == JBLK else (outn_pool, "obn")
            ob = ob_pool.tile([P, width], out_dt, tag=tag, name=tag)
            # products are +/-0.5 * +/-0.5 = +/-0.25 -> scale by 4
            nc.vector.tensor_scalar_mul(ob[:], ps[:], 4.0)
            nc.scalar.dma_start(
                out[ib * P:(ib + 1) * P, jc * width:(jc + 1) * width], ob[:]
            )

        def flush_out():
            while outq:
                emit_out(*outq.pop(0))

        def mm(ib, jc):
            require(("x", ib))
            for s in range(JB_PER_JC):
                require(("w", jc * JB_PER_JC + s))
            ps = psum_mm.tile([P, JBLK], f32, tag="ps", name="ps")
            for kp in range(KT16):
                lhsT = xbT[ib][:, kp, :].bitcast(fp8)
                rhs = wbT[jc][:, kp, :].bitcast(fp8).rearrange(
                    "p (n two) -> p two n", two=2)
                nc.tensor.matmul(ps[:], lhsT=lhsT, rhs=rhs,
                                 start=(kp == 0), stop=(kp == KT16 - 1),
                                 perf_mode=DR)
                pump(1, hold=8)
            if outq:
                emit_out(*outq.pop(0))
            outq.append((ps, ib, jc, JBLK))

        def narrow_mm(ib, jb):
            require(("x", ib))
            require(("w", jb))
            # same tag/shape as wide sets (pool bufs are per-tag); narrow
            # output lives in the first quarter of a full-width tile
            ps = psum_mm.tile([P, JBLK], f32, tag="ps", name="ps")[:, :P]
            for kp in range(KT16):
                lhsT = xbT[ib][:, kp, :].bitcast(fp8)
                rhs = wbT[0][:, kp, jb * P:(jb + 1) * P].bitcast(fp8).rearrange(
                    "p (n two) -> p two n", two=2)
                nc.tensor.matmul(ps[:], lhsT=lhsT, rhs=rhs,
                                 start=(kp == 0), stop=(kp == KT16 - 1),
                                 perf_mode=DR)
                pump(1, hold=8)
            if outq:
                emit_out(*outq.pop(0))
            outq.append((ps, ib, jb, P))

        # ---- startup: minimal front-load; first narrow set needs only
        # w-jb0 + x0 binarized + transposed (~4MB landed).
        prep_w_jb(0)
        prep_x(0)
        prep_w_jb(1)
        prep_w_jb(2)
        prep_w_jb(3)
        prep_x(1)
        # Drain startup transpose groups with warm filler to hold the PE
        # clock gate open while the loads land.
        n_drain = len(pending)
        for i in range(n_drain):
            pump(1)
            if warmup:
                warm(3)
        for ib in (0, 1):
            for jb in range(JB_PER_JC):
                narrow_mm(ib, jb)
                if warmup and ib == 0:
                    warm(2)
        # ---- solo phase: x-arrival paced, chunk1 streaming behind.
        for ib in range(2, 6):
            advance_loads(2)
            mm(ib, 0)
        # ---- pair phase.
        for ib in range(6, IB):
            advance_loads(2)
            mm(ib, 0)
            advance_loads(1)
            mm(ib, 1)
        # ---- debt: owed (0..5, 1).
        for ib in range(6):
            advance_loads(2)
            mm(ib, 1)
        # ---- jc 2/3 sweeps; finish remaining loads early in jc2.
        for jc in range(2, JC):
            for ib in range(IB):
                advance_loads(2)
                mm(ib, jc)
        flush_out()
        pump(len(pending))

    nc.compile()
    _PROGRAM_CACHE[key] = nc
    return nc


def kernel(x, weight):
    x = np.ascontiguousarray(np.asarray(x), dtype=np.float32)
    w = np.ascontiguousarray(np.asarray(weight), dtype=np.float32)
    assert x.shape == (FULL_M, FULL_K) and w.shape == (FULL_N, FULL_K)

    from concourse.bass_utils import run_bass_kernel_spmd

    nc = build_program()
    in_maps = []
    for c in range(N_CORES):
        r, s = divmod(c, GRID_J)
        in_maps.append({
            "xs": x[r * M_SH:(r + 1) * M_SH],
            "ws": w[s * N_SH:(s + 1) * N_SH],
        })
    res = run_bass_kernel_spmd(nc, in_maps, core_ids=list(range(N_CORES))).results
    outp = np.empty((FULL_M, FULL_N), dtype=np.float32)
    for c in range(N_CORES):
        r, s = divmod(c, GRID_J)
        blk = np.asarray(res[c]["out"], dtype=np.float32)
        # SwInterleave reads the stationary m axis reversed: un-flip each
        # 128-row output block.
        blk = blk.reshape(M_SH // P, P, N_SH)[:, ::-1, :].reshape(M_SH, N_SH)
        outp[r * M_SH:(r + 1) * M_SH, s * N_SH:(s + 1) * N_SH] = blk
    return outp


# revision 14
# speedup vs baseline: 1.0968x; 1.0020x over previous
"""Binarized linear: out = sign(x+eps) @ sign(w+eps).T on 8 trn2 cores.

Sharding: 4x2 grid. Core c=(r,s): rows x[r*2048:(r+1)*2048], rows
w[s*2048:(s+1)*2048]. Each core computes a [2048, 2048] output block; the
host concatenates. No collectives.

Per-core kernel (all arithmetic exact -> rel err 0 vs the f32 reference):
  - binarize BOTH operands to fp8e4m3 +/-0.5 on DVE ((v>=0)-0.5, matching
    sign(v+1e-20) away from the measure-zero region f32 randn never hits).
    Products are +/-0.25 -> output eviction scales PSUM by 4: exact even
    integers |v|<=4096, stored f16 (exact; host casts back to f32).
  - transpose to [K-on-partition] layout via PE is_transpose matmuls on
    fp16 PAIRS (two adjacent fp8 k-values ride one 16-bit lane; all our
    fp8 byte pairs form normal f16 values so the move is bit-exact), TG=4
    tiles per PSUM group, one contiguous f16 eviction per group (ACT).
    (A DMA-XBAR variant was measured: transposes inflate 3->9us when
    competing with input loads for the 16 DMA engines, and the shared
    DMA-semaphore rotation couples input loads to XBAR completions,
    collapsing supply to ~250GB/s. PE transposes pump into the DR stream's
    inter-pass bubbles at ~110ns net each instead.)
  - DR matmuls run perf_mode=DoubleRowSwInterleave on the interleaved pair
    layout (hw deinterleaves; reads the stationary m axis reversed -- host
    un-flips each 128-row output block). Measured steady cadence: 259ns
    per K=256 x 512-wide pass; the ~46ns/pass gap vs 213ns streaming
    theory is a fixed per-instruction bubble (PSUM-bank alternation and
    2-bank-wide outputs measured/rejected: no effect / ISA-illegal).
  - floors: PE 1024 DR passes x 259ns = 265us/core + ~35-55us net
    transposes; DMA 64MB input at ~380GB/s one-queue serial (16 engines
    x ~24GB/s each, saturated by a single queue's 16KB-row descriptors).

Queues: SP (sync) = input loads only, one 2MB full-row DMA per 128-row
block, in arrival order: w-jb0, x0, w-jb1..3, x1, then x blocks 1:1 with
w jbs. ACT (scalar) = transpose-group evictions + output stores.

Schedule ("debt" order -- first matmul after only ~4MB has landed, vs 20MB
for the v1 two-phase schedule, and supply stays under the ~380GB/s ceiling
in every phase, where v1's x-streaming phase demanded ~360+):
  - narrow phase (~t=16us): 8 n=128 pass-sets (ib 0/1 x jb 0..3) as soon
    as w-jb0+x0 are transposed, covering (ib0,jc0)+(ib1,jc0). Warm
    matmuls (no-dep identity passes) pad the PE clock gate open.
  - solo: mm(ib,0) ib 2..5 (x-arrival paced) while chunk1 lands.
  - pairs: mm(ib,0)+mm(ib,1) ib 6..15; chunks 2/3 stream behind.
  - debt: owed mm(0..5,1), then jc=2, jc=3 sweeps (all resident).
  - transpose groups pump one per DR pass; out evictions (DVE) are
    emitted one set late so their PE-completion waits never
    head-of-line-block the DVE queue (which also binarizes).
The Tile scheduler is fed PE timings scaled 2x (build_program patches
TRN2Spec) because the stock cost model prices DR fp8 matmuls at half
their measured hardware cost.
"""

from collections import deque

import numpy as np

P = 128
GRID_I, GRID_J = 4, 2
N_CORES = 8
FULL_M, FULL_N, FULL_K = 8192, 4096, 4096
M_SH, N_SH = FULL_M // GRID_I, FULL_N // GRID_J  # 2048, 2048

_PROGRAM_CACHE = {}


def build_program(m_sh=M_SH, n_sh=N_SH, k=FULL_K, warmup=64, out_fp16=True):
    """Build (and cache) the per-core Bass program. Same SPMD program on all cores."""
    key = (m_sh, n_sh, k, warmup, out_fp16)
    if key in _PROGRAM_CACHE:
        return _PROGRAM_CACHE[key]

    from contextlib import ExitStack

    import concourse.bass as bass
    import concourse.mybir as mybir
    from concourse import bacc, tile
    from concourse.masks import make_identity

    # Feed the Tile scheduler PE timings that match measured hw (stock model
    # prices DR fp8 at 0.5 cyc/row; hw runs ~1.21 cyc/row incl the bubble).
    from concourse import hw_specs as _hw
    _hw.TRN2Spec.PE_CYCLE = 2.0 / 2.4
    _hw.TRN2Spec.PE_CYCLE_PSTATE_MID = 2.0 / 1.2
    _hw.TRN2Spec.PE_CYCLE_PSTATE_LOW = 2.0 / 0.65

    f32 = mybir.dt.float32
    f16 = mybir.dt.float16
    fp8 = mybir.dt.float8e4
    out_dt = f16 if out_fp16 else f32

    KT16 = k // (2 * P)  # 128-wide f16-pair k tiles (16)
    IB = m_sh // P       # 16 x blocks
    JBLK = 512
    JC = n_sh // JBLK    # 4 w chunks
    JB = n_sh // P       # 16 w j-blocks
    JB_PER_JC = JBLK // P
    TG = 4               # f16 tiles per transpose-evict group
    assert KT16 % TG == 0

    DR = mybir.MatmulPerfMode.DoubleRowSwInterleave

    nc = bacc.Bacc("TRN2", target_bir_lowering=False, debug=False)
    xs = nc.dram_tensor("xs", [m_sh, k], f32, kind="ExternalInput").ap()
    ws = nc.dram_tensor("ws", [n_sh, k], f32, kind="ExternalInput").ap()
    out = nc.dram_tensor("out", [m_sh, n_sh], out_dt, kind="ExternalOutput").ap()

    with tile.TileContext(nc) as tc, ExitStack() as ctx:
        const_pool = ctx.enter_context(tc.tile_pool(name="const", bufs=1))
        stage = ctx.enter_context(tc.tile_pool(name="stage", bufs=3))
        xb8_pool = ctx.enter_context(tc.tile_pool(name="xb8", bufs=2))
        wb8_pool = ctx.enter_context(tc.tile_pool(name="wb8", bufs=2))
        xbt_pool = ctx.enter_context(tc.tile_pool(name="xbt", bufs=1))
        wbt_pool = ctx.enter_context(tc.tile_pool(name="wbt", bufs=1))
        out_pool = ctx.enter_context(tc.tile_pool(name="outp", bufs=3))
        outn_pool = ctx.enter_context(tc.tile_pool(name="outn", bufs=2))
        psum_t = ctx.enter_context(tc.tile_pool(name="pst", bufs=3, space="PSUM"))
        psum_mm = ctx.enter_context(tc.tile_pool(name="psmm", bufs=4, space="PSUM"))
        psum_wu = ctx.enter_context(tc.tile_pool(name="pswu", bufs=1, space="PSUM"))

        ident = const_pool.tile([P, P], fp8, tag="ident")
        make_identity(nc, ident)
        ident16 = const_pool.tile([P, P], f16, tag="ident16")
        nc.vector.tensor_copy(ident16[:], ident[:])

        warm_psum = psum_wu.tile([P, P], f32, tag="warm", name="warm") if warmup else None

        def warm(n):
            for _ in range(n):
                nc.tensor.matmul(warm_psum[:], lhsT=ident[:], rhs=ident[:],
                                 start=True, stop=True)

        if warmup:
            warm(warmup)

        # Resident transposed binarized operands, f16 pair layout, K on
        # partitions: f16 k-tile kp, partition p, byte b holds fp8 k value
        # 256*kp + 2p + b (consistent across x and w).
        xbT = [
            xbt_pool.tile([P, KT16, P], f16, tag=f"xbt{ib}", name=f"xbt{ib}")
            for ib in range(IB)
        ]
        wbT = [
            wbt_pool.tile([P, KT16, JBLK], f16, tag=f"wbt{jc}", name=f"wbt{jc}")
            for jc in range(JC)
        ]

        def binarize(b8h, stgh):
            # (v >= 0) -> {1,0}; minus 0.5 -> +/-0.5.
            nc.vector.tensor_scalar(
                b8h, stgh, 0.0, 0.5,
                mybir.AluOpType.is_ge, mybir.AluOpType.subtract,
            )

        pending = deque()  # (key, closure): transpose groups, 4 PE transposes each
        pending_cnt = {}   # key -> groups still pending

        def queue_tgroups(b8, dest, dest_col0, key):
            """Transpose b8 (fp8 [128, k]) into dest[:, :, col0:col0+P] (f16
            pair layout) via fp16-pair PE transposes + one contiguous f16
            eviction per group (ACT)."""
            for g in range(KT16 // TG):
                def go(g=g, b8=b8, dest=dest, dest_col0=dest_col0):
                    pt = psum_t.tile([P, TG, P], f16, tag="pt", name="pt")
                    for t in range(TG):
                        t16 = g * TG + t
                        in16 = b8[:, 2 * P * t16:2 * P * (t16 + 1)].bitcast(f16)
                        nc.tensor.transpose(pt[:, t, :], in16, ident16[:])
                    # alternate eviction engine so consecutive groups never
                    # wait on the same engine's queue
                    evict = nc.scalar.copy if g % 2 == 0 else nc.vector.tensor_copy
                    evict(
                        dest[:, TG * g:TG * (g + 1), dest_col0:dest_col0 + P],
                        pt[:])
                pending.append((key, go))
                pending_cnt[key] = pending_cnt.get(key, 0) + 1


        def pump(n, hold=0):
            """Emit up to n pending transpose groups (oldest first); with
            hold>0, keep at least that many queued (so a pumped transpose's
            input load was issued several blocks back and its data-wait
            never stalls the PE FIFO)."""
            for _ in range(n):
                if len(pending) <= hold:
                    return
                key, go = pending.popleft()
                pending_cnt[key] -= 1
                go()

        prepped = set()

        def require(key):
            """Emit all transpose groups up to and including key's (FIFO
            order matches need order). Must precede any matmul reading the
            operand tile that key's groups write."""
            assert key in prepped, f"operand {key} never prepped"
            while pending_cnt.get(key, 0) > 0:
                key2, go = pending.popleft()
                pending_cnt[key2] -= 1
                go()

        def prep_x(ib):
            stg = stage.tile([P, k], f32, tag="stg", name="stg")
            nc.sync.dma_start(stg[:], xs[ib * P:(ib + 1) * P, :])
            b8 = xb8_pool.tile([P, k], fp8, tag="xb8", name="xb8")
            binarize(b8[:], stg[:])
            queue_tgroups(b8, xbT[ib], 0, ("x", ib))
            prepped.add(("x", ib))

        def prep_w_jb(jb):
            jc, sub = divmod(jb, JB_PER_JC)
            stg = stage.tile([P, k], f32, tag="stg", name="stg")
            nc.sync.dma_start(stg[:], ws[jb * P:(jb + 1) * P, :])
            b8 = wb8_pool.tile([P, k], fp8, tag="wb8", name="wb8")
            binarize(b8[:], stg[:])
            queue_tgroups(b8, wbT[jc], sub * P, ("w", jb))
            prepped.add(("w", jb))

        # ---- load pacing: interleave remaining x blocks and w jbs ----
        load_q = []
        for i in range(2, IB):
            load_q.append(("x", i))
            if 2 + i < JB:
                load_q.append(("w", 2 + i))
        for j in range(2 + IB, JB):
            load_q.append(("w", j))

        def advance_loads(n):
            for _ in range(n):
                if not load_q:
                    return
                kind, idx = load_q.pop(0)
                if kind == "x":
                    prep_x(idx)
                else:
                    prep_w_jb(idx)

        outq = []

        def emit_out(ps, ib, jc, width):
            ob_pool, tag = (out_pool, "ob") if width == JBLK else (outn_pool, "obn")
            ob = ob_pool.tile([P, width], out_dt, tag=tag, name=tag)
            # products are +/-0.5 * +/-0.5 = +/-0.25 -> scale by 4
            nc.vector.tensor_scalar_mul(ob[:], ps[:], 4.0)
            nc.scalar.dma_start(
                out[ib * P:(ib + 1) * P, jc * width:(jc + 1) * width], ob[:]
            )

        def flush_out():
            while outq:
                emit_out(*outq.pop(0))

        def mm(ib, jc):
            require(("x", ib))
            for s in range(JB_PER_JC):
                require(("w", jc * JB_PER_JC + s))
            ps = psum_mm.tile([P, JBLK], f32, tag="ps", name="ps")
            for kp in range(KT16):
                lhsT = xbT[ib][:, kp, :].bitcast(fp8)
                rhs = wbT[jc][:, kp, :].bitcast(fp8).rearrange(
                    "p (n two) -> p two n", two=2)
                nc.tensor.matmul(ps[:], lhsT=lhsT, rhs=rhs,
                                 start=(kp == 0), stop=(kp == KT16 - 1),
                                 perf_mode=DR)
                pump(1, hold=4)
            if outq:
                emit_out(*outq.pop(0))
            outq.append((ps, ib, jc, JBLK))

        def narrow_mm(ib, jb):
            require(("x", ib))
            require(("w", jb))
            # same tag/shape as wide sets (pool bufs are per-tag); narrow
            # output lives in the first quarter of a full-width tile
            ps = psum_mm.tile([P, JBLK], f32, tag="ps", name="ps")[:, :P]
            for kp in range(KT16):
                lhsT = xbT[ib][:, kp, :].bitcast(fp8)
                rhs = wbT[0][:, kp, jb * P:(jb + 1) * P].bitcast(fp8).rearrange(
                    "p (n two) -> p two n", two=2)
                nc.tensor.matmul(ps[:], lhsT=lhsT, rhs=rhs,
                                 start=(kp == 0), stop=(kp == KT16 - 1),
                                 perf_mode=DR)
                pump(1, hold=4)
            if outq:
                emit_out(*outq.pop(0))
            outq.append((ps, ib, jb, P))

        # ---- startup: minimal front-load; first narrow set needs only
        # w-jb0 + x0 binarized + transposed (~4MB landed).
        prep_w_jb(0)
        prep_x(0)
        prep_w_jb(1)
        prep_w_jb(2)
        prep_w_jb(3)
        prep_x(1)
        # Drain startup transpose groups with warm filler to hold the PE
        # clock gate open while the loads land.
        n_drain = len(pending)
        for i in range(n_drain):
            pump(1)
            if warmup:
                warm(2)
        for ib in (0, 1):
            for jb in range(JB_PER_JC):
                narrow_mm(ib, jb)
                if warmup and ib == 0:
                    warm(2)
        # ---- solo phase: x-arrival paced, chunk1 streaming behind.
        for ib in range(2, 6):
            advance_loads(2)
            mm(ib, 0)
        # ---- pair phase.
        for ib in range(6, IB):
            advance_loads(2)
            mm(ib, 0)
            advance_loads(1)
            mm(ib, 1)
        # ---- debt: owed (0..5, 1).
        for ib in range(6):
            advance_loads(2)
            mm(ib, 1)
        # ---- jc 2/3 sweeps; finish remaining loads early in jc2.
        for jc in range(2, JC):
            for ib in range(IB):
                advance_loads(2)
                mm(ib, jc)
        flush_out()
        pump(len(pending))

    nc.compile()
    _PROGRAM_CACHE[key] = nc
    return nc


def kernel(x, weight):
    x = np.ascontiguousarray(np.asarray(x), dtype=np.float32)
    w = np.ascontiguousarray(np.asarray(weight), dtype=np.float32)
    assert x.shape == (FULL_M, FULL_K) and w.shape == (FULL_N, FULL_K)

    from concourse.bass_utils import run_bass_kernel_spmd

    nc = build_program()
    in_maps = []
    for c in range(N_CORES):
        r, s = divmod(c, GRID_J)
        in_maps.append({
            "xs": x[r * M_SH:(r + 1) * M_SH],
            "ws": w[s * N_SH:(s + 1) * N_SH],
        })
    res = run_bass_kernel_spmd(nc, in_maps, core_ids=list(range(N_CORES))).results
    outp = np.empty((FULL_M, FULL_N), dtype=np.float32)
    for c in range(N_CORES):
        r, s = divmod(c, GRID_J)
        blk = np.asarray(res[c]["out"], dtype=np.float32)
        # SwInterleave reads the stationary m axis reversed: un-flip each
        # 128-row output block.
        blk = blk.reshape(M_SH // P, P, N_SH)[:, ::-1, :].reshape(M_SH, N_SH)
        outp[r * M_SH:(r + 1) * M_SH, s * N_SH:(s + 1) * N_SH] = blk
    return outp
